# revision 103
# baseline (speedup 1.0000x reference)
"""Trainium2 Bass kernel for nn_Attention (channel attention, XCA-style).

Sharding: 8 cores = (batch b=core//2) x (image half = core%2, 64 rows + halo).
Cross-core: AllGather of tiny gram stats over core pairs + local add.

All heavy matmuls run in fp8e4m3 with DoubleRow (2 contraction planes per
instruction, 0.5 cycles/row): conv q,k,v (channel planes), transposed
depthwise (vertical tap pairs, 16-aligned via WS=144), gram (pixel-chunk
pairs), attn 9-tap folds (tap pairs), proj (attn-channel planes).
"""

import sys
import numpy as np

sys.path.insert(0, "/opt/trn_rl_repo")

import contextlib  # noqa: E402

import ml_dtypes  # noqa: E402

from concourse import bass, bacc, tile, mybir  # noqa: E402
from concourse import bass_utils  # noqa: E402

F32 = mybir.dt.float32
BF16 = mybir.dt.bfloat16
F8 = mybir.dt.float8e4
ALU = mybir.AluOpType
ACTF = mybir.ActivationFunctionType
AX = mybir.AxisListType
DR = mybir.MatmulPerfMode.DoubleRow
BF16NP = ml_dtypes.bfloat16
F8NP = ml_dtypes.float8_e4m3

C = 192
HEADS = 8
CH = 24
W = 128
HOUT = 64
HIN = HOUT + 2
WS = 144                  # padded row stride (16-aligned for DoubleRow)
PXIN = HIN * W            # 8448
PXOUT = HOUT * W          # 8192

RS = 16                   # stripe out-rows
NS = HOUT // RS
RIN = RS + 2
SPXI = RIN * W            # 2304
SPXO = RS * W             # 2048
LT = RIN * WS             # padded buffer length (2592)
MMCH = 512

TAPS = [(dy, dx) for dy in (0, 1, 2) for dx in (0, 1, 2)]
# DoubleRow tap pairs must have 16-aligned flat-offset delta: vertical
# pairs (same dx) have delta WS=144. Taps 6,7,8 run as single fp8 MMs.
TAP_PAIRS = [(0, 3), (1, 4), (2, 5)]
TAP_SINGLES = [6, 7, 8]

_CACHE = {}


def _chunks(total, step):
    out, s = [], 0
    while s < total:
        out.append((s, min(step, total - s)))
        s += step
    return out


def _tap_off(cix, ti):
    dy, dx = TAPS[ti]
    return (cix + dy) * WS + 1 + dx


def pair_view(flat, cix, ti, tj):
    """[P, 2, 128] view of two tap windows (plane stride = o1-o0)."""
    o0, o1 = _tap_off(cix, ti), _tap_off(cix, tj)
    d = o1 - o0
    v = flat[:, o0:o0 + 2 * d].rearrange("p (two d) -> p two d", d=d)
    return v[:, :, 0:128]


def wide_pair(wtile, ti, tj, blkw=128):
    """[P, 2, 128] view of two tap blocks in a [P, 9*blkw] weight tile."""
    o0, d = ti * blkw, (tj - ti) * blkw
    v = wtile[:, o0:o0 + 2 * d].rearrange("p (two d) -> p two d", d=d)
    return v[:, :, 0:128]


def build_program():
    nc = bacc.Bacc("TRN2", target_bir_lowering=False, debug=False,
                   enable_asserts=False, num_devices=8)
    io = {}
    io["xs8"] = nc.dram_tensor("xs8", [128, 2 * PXIN], F8,
                               kind="ExternalInput").ap()
    io["ys8"] = nc.dram_tensor("ys8", [128, 2 * PXIN], F8,
                               kind="ExternalInput").ap()
    io["wq8"] = nc.dram_tensor("wq8", [128, 2 * C], F8,
                               kind="ExternalInput").ap()
    io["wk8"] = nc.dram_tensor("wk8", [128, 2 * C], F8,
                               kind="ExternalInput").ap()
    io["xs"] = nc.dram_tensor("xs", [C, PXIN], BF16, kind="ExternalInput").ap()
    io["wv"] = nc.dram_tensor("wv", [C, C], BF16, kind="ExternalInput").ap()
    io["wp1"] = nc.dram_tensor("wp1", [120, C], BF16,
                               kind="ExternalInput").ap()
    io["wp2"] = nc.dram_tensor("wp2", [72, C], BF16,
                               kind="ExternalInput").ap()
    io["dqkd"] = nc.dram_tensor("dqkd", [3 * 128, 9 * 128], F8,
                                kind="ExternalInput").ap()
    io["dvw"] = nc.dram_tensor("dvw", [C, 9], F32, kind="ExternalInput").ap()
    io["tmpq"] = nc.dram_tensor("tmpq", [C, 1], F32, kind="ExternalInput").ap()
    io["em"] = nc.dram_tensor("em", [HEADS, C], F32, kind="ExternalInput").ap()
    io["eye"] = nc.dram_tensor("eye", [128, 128], F32,
                               kind="ExternalInput").ap()
    io["outp"] = nc.dram_tensor("outp", [C, PXOUT], F32,
                                kind="ExternalOutput").ap()

    with tile.TileContext(nc) as tc, contextlib.ExitStack() as es:
        _emit(nc, tc, io, es)
    nc.compile()
    return nc


def _emit(nc, tc, io, es):
    # ---------------- persistent weights ------------------------------
    wpool = es.enter_context(tc.tile_pool(name="w", bufs=1))
    wq8_t = wpool.tile([128, 2 * C], F8, tag="wq8")
    wk8_t = wpool.tile([128, 2 * C], F8, tag="wk8")
    nc.sync.dma_start(wq8_t[:], io["wq8"][:])
    nc.sync.dma_start(wk8_t[:], io["wk8"][:])
    wv_a = wpool.tile([128, C], BF16, tag="wva")
    wv_b = wpool.tile([64, C], BF16, tag="wvb")
    wp1_t = wpool.tile([120, C], BF16, tag="wp1")
    wp2_t = wpool.tile([72, C], BF16, tag="wp2")
    with tc.tile_wait_until(0.010):
        nc.sync.dma_start(wv_a[:], io["wv"][0:128, :])
        nc.sync.dma_start(wv_b[:], io["wv"][128:192, :])
        nc.sync.dma_start(wp1_t[:], io["wp1"][:])
        nc.sync.dma_start(wp2_t[:], io["wp2"][:])
    dqkd_t = [wpool.tile([128, 9 * 128], F8, tag=f"dqkd{i}",
                         name=f"dqkd{i}") for i in range(3)]
    for i in range(3):
        nc.sync.dma_start(dqkd_t[i][:], io["dqkd"][i * 128:(i + 1) * 128, :])
    dvw1_t = wpool.tile([120, 9], F32, tag="dvw1")
    dvw2_t = wpool.tile([72, 9], F32, tag="dvw2")
    tmpq_t = wpool.tile([128, 2], F32, tag="tmpq")
    em_t = wpool.tile([HEADS, C], F32, tag="em")
    eye_t = wpool.tile([128, 128], F32, tag="eye")
    with tc.tile_wait_until(0.010):
        nc.sync.dma_start(dvw1_t[:], io["dvw"][0:120, :])
        nc.sync.dma_start(dvw2_t[:], io["dvw"][120:192, :])
        nc.sync.dma_start(tmpq_t[:, 0:1], io["tmpq"][0:128, :])
        nc.sync.dma_start(tmpq_t[0:64, 1:2], io["tmpq"][128:192, :])
        nc.sync.dma_start(em_t[:], io["em"][:])
        nc.sync.dma_start(eye_t[:], io["eye"][:])

    # ---------------- pools -------------------------------------------
    inb = es.enter_context(tc.tile_pool(name="inb", bufs=1))
    tbuf = es.enter_context(tc.tile_pool(name="tbuf", bufs=1))
    dwo = es.enter_context(tc.tile_pool(name="dwo", bufs=1))
    stck = es.enter_context(tc.tile_pool(name="stck", bufs=1))
    small = es.enter_context(tc.tile_pool(name="small", bufs=1))
    outsb = es.enter_context(tc.tile_pool(name="outsb", bufs=2))
    drm = es.enter_context(tc.tile_pool(name="drm", bufs=1, space="DRAM"))

    # pre-zero only the pad columns of each padded buffer slot (cols 0:2
    # and 130:132 of every row; data region is overwritten each stripe;
    # cols 132:144 are never read)
    for b in range(3):
        for _sl in range(2):
            tz = tbuf.tile([128, LT], F8, tag=f"t{b}", name=f"tz{b}_{_sl}",
                           bufs=2)
            tzv = tz[:].rearrange("p (r w) -> p r w", w=WS)
            nc.vector.memset(tzv[:, :, 0:2], 0.0)
            nc.vector.memset(tzv[:, :, 130:132], 0.0)
    tv_tiles = {}
    for _s in range(NS):
        _tva = tbuf.tile([120, LT], BF16, tag="tv_{}".format(_s),
                         name=f"tvz_{_s}", bufs=1)
        _tvb = tbuf.tile([72, LT], BF16, tag="tvb_{}".format(_s),
                         name=f"tvbz_{_s}", bufs=1)
        for _t in (_tva, _tvb):
            _tvv = _t[:].rearrange("p (r w) -> p r w", w=WS)
            nc.vector.memset(_tvv[:, :, 0:2], 0.0)
            nc.vector.memset(_tvv[:, :, 130:132], 0.0)

    xs8v = io["xs8"].rearrange("p (two n) -> p two n", two=2)
    ys8v = io["ys8"].rearrange("p (two n) -> p two n", two=2)
    wq8v = wq8_t[:].rearrange("p (two c) -> p two c", two=2)
    wk8v = wk8_t[:].rearrange("p (two c) -> p two c", two=2)

    # ================= PASS 1 =========================================
    # gram PSUM layout:
    #   gA  = [selfg0 (128) | b1 self (128) | selfg2 (128)]  (diag -> norms)
    #   hAB = per-head q x k blocks at partition base 0:
    #         [h0..h4 (5x24) | h5-main 8 rows (24) | h6 (24) | h7 (24) |
    #          h5-aux 16 rows (24)]
    x_tiles = {}
    with tc.tile_pool(name="cps", bufs=1, space="PSUM") as cpsum, \
         tc.tile_pool(name="gps", bufs=1, space="PSUM") as gpsum:
        gA_ps = gpsum.tile([128, 384], F32, tag="gA")
        hAB_ps = gpsum.tile([24, 216], F32, tag="hAB")

        def _qpieces(h):          # (stk idx, lo, hi, row_off)
            q0 = 24 * h
            if q0 + 24 <= 128:
                return [(0, q0, q0 + 24, 0)]
            if q0 >= 128:
                return [(1, q0 - 128, q0 - 104, 0)]
            return [(0, q0, 128, 0), (1, 0, q0 - 104, 128 - q0)]

        def _kpieces(h):          # (stk idx, lo, hi, col_off)
            k0 = 24 * h
            if k0 + 24 <= 64:
                return [(1, 64 + k0, 88 + k0, 0)]
            if k0 >= 64:
                return [(2, k0 - 64, k0 - 40, 0)]
            return [(1, 64 + k0, 128, 0), (2, 0, k0 - 40, 64 - k0)]

        def _hout(h, roff, rs, coff, cs):
            if h <= 4:
                c0 = 24 * h
            elif h == 5:
                c0 = 120 if roff == 0 else 192
            else:
                c0 = 120 + 24 * (h - 5)
            return hAB_ps[0:rs, c0 + coff:c0 + coff + cs]

        for s in range(NS):
            i0 = s * RS * W
            t_blk = [tbuf.tile([128, LT], F8, tag=f"t{b}",
                               name=f"t{b}_{s}", bufs=2) for b in range(3)]

            xq8 = inb.tile([128, 2 * SPXI], F8, tag="xq8", bufs=2)
            yq8 = inb.tile([128, 2 * SPXI], F8, tag="yq8", bufs=2)
            xa16 = inb.tile([128, SPXI], BF16, tag=f"xa16_{s}", bufs=1)
            xb16 = inb.tile([64, SPXI], BF16, tag=f"xb16_{s}", bufs=1)
            x_tiles[s] = (xa16, xb16)
            nc.gpsimd.dma_start(
                yq8[:].rearrange("p (two n) -> p two n", two=2),
                ys8v[:, :, i0:i0 + SPXI])
            nc.gpsimd.dma_start(
                xq8[:].rearrange("p (two n) -> p two n", two=2),
                xs8v[:, :, i0:i0 + SPXI])
            nc.gpsimd.dma_start(xa16[:], io["xs"][0:128, i0:i0 + SPXI])
            nc.gpsimd.dma_start(xb16[:], io["xs"][128:192, i0:i0 + SPXI])
            xqv = xq8[:].rearrange("p (two n) -> p two n", two=2)
            yqv = yq8[:].rearrange("p (two n) -> p two n", two=2)

            # conv q,k (fp8 DR over the 192-channel contraction), v (bf16)
            # t blocks: [q 0:128] | [q 128:192 ; k 0:64] | [k 64:192]
            tv_a = tbuf.tile([120, LT], BF16, tag="tv_{}".format(s),
                             name=f"tva_{s}", bufs=1)
            tv_b = tbuf.tile([72, LT], BF16, tag="tvb_{}".format(s),
                             name=f"tvb_{s}", bufs=1)
            tv_tiles[s] = (tv_a, tv_b)
            for n0, n in _chunks(SPXI, MMCH):
                r0, nr = n0 // W, n // W
                ps0 = cpsum.tile([128, MMCH], F32, tag="cps0", bufs=2)
                ps1 = cpsum.tile([128, MMCH], F32, tag="cps1", bufs=2)
                psk = cpsum.tile([128, MMCH], F32, tag="cps0", bufs=2,
                                 name=f"psk_{s}_{n0}")
                ps2 = cpsum.tile([128, MMCH], F32, tag="cps1", bufs=2,
                                 name=f"ps2_{s}_{n0}")
                nc.tensor.matmul(ps0[:, 0:n], wq8v[:, :, 0:128],
                                 yqv[:, :, n0:n0 + n], start=True, stop=True,
                                 perf_mode=DR)
                nc.tensor.matmul(ps1[0:64, 0:n], wq8v[:, :, 128:192],
                                 yqv[:, :, n0:n0 + n], start=True, stop=True,
                                 perf_mode=DR)
                nc.tensor.matmul(psk[0:64, 0:n], wk8v[:, :, 0:64],
                                 xqv[:, :, n0:n0 + n], start=True, stop=True,
                                 perf_mode=DR)
                nc.tensor.matmul(ps2[:, 0:n], wk8v[:, :, 64:192],
                                 xqv[:, :, n0:n0 + n], start=True, stop=True,
                                 perf_mode=DR)
                t0d = t_blk[0][:].rearrange("p (r w) -> p r w", w=WS)
                t1d = t_blk[1][:].rearrange("p (r w) -> p r w", w=WS)
                t2d = t_blk[2][:].rearrange("p (r w) -> p r w", w=WS)

                def pw(ps, lo, hi):
                    return ps[lo:hi, 0:n].rearrange("p (r w) -> p r w", w=W)

                nc.scalar.copy(t0d[:, r0:r0 + nr, 2:130], pw(ps0, 0, 128))
                nc.vector.tensor_copy(t1d[0:64, r0:r0 + nr, 2:130],
                                      pw(ps1, 0, 64))
                nc.scalar.copy(t1d[64:128, r0:r0 + nr, 2:130],
                               pw(psk, 0, 64))
                nc.scalar.copy(t2d[:, r0:r0 + nr, 2:130], pw(ps2, 0, 128))
                if s < 2:
                    cv0 = cpsum.tile([128, MMCH], F32, tag="cps0", bufs=2,
                                     name=f"cv0_{s}_{n0}")
                    cv1 = cpsum.tile([128, MMCH], F32, tag="cps1", bufs=2,
                                     name=f"cv1_{s}_{n0}")
                    nc.tensor.matmul(cv0[0:120, 0:n], wv_a[:, 0:120],
                                     xa16[:, n0:n0 + n], start=True,
                                     stop=False)
                    nc.tensor.matmul(cv0[0:120, 0:n], wv_b[:, 0:120],
                                     xb16[:, n0:n0 + n], start=False,
                                     stop=True)
                    nc.tensor.matmul(cv1[0:72, 0:n], wv_a[:, 120:192],
                                     xa16[:, n0:n0 + n], start=True,
                                     stop=False)
                    nc.tensor.matmul(cv1[0:72, 0:n], wv_b[:, 120:192],
                                     xb16[:, n0:n0 + n], start=False,
                                     stop=True)
                    tvad = tv_a[:].rearrange("p (r w) -> p r w", w=WS)
                    tvbd = tv_b[:].rearrange("p (r w) -> p r w", w=WS)
                    nc.scalar.copy(tvad[0:120, r0:r0 + nr, 2:130],
                                   pw(cv0, 0, 120))
                    nc.vector.tensor_copy(tvbd[0:72, r0:r0 + nr, 2:130],
                                          pw(cv1, 0, 72))

            # transposed depthwise on PE (fp8 DR tap pairs) -> px-major
            # stacks, then gram accumulation (fp8 DR chunk pairs)
            nchunk = SPXO // 128
            for g in range(nchunk // 4):
                stk = [stck.tile([128, MMCH], F8, tag=f"stk{b}",
                                 name=f"stk{b}_{s}_{g}", bufs=2)
                       for b in range(3)]
                for b in range(3):
                    tp = cpsum.tile([128, MMCH], F32, tag="tps",
                                    name=f"tp{b}_{s}_{g}", bufs=2)
                    tflat = t_blk[b][:]
                    for ci in range(4):
                        cix = g * 4 + ci
                        q0 = ci * 128
                        for pi, (ti, tj) in enumerate(TAP_PAIRS):
                            nc.tensor.matmul(
                                tp[:, q0:q0 + 128],
                                pair_view(tflat, cix, ti, tj),
                                wide_pair(dqkd_t[b], ti, tj),
                                start=(pi == 0), stop=False, perf_mode=DR)
                        for si, ti in enumerate(TAP_SINGLES):
                            o8 = _tap_off(cix, ti)
                            nc.tensor.matmul(
                                tp[:, q0:q0 + 128], tflat[:, o8:o8 + 128],
                                dqkd_t[b][:, ti * 128:(ti + 1) * 128],
                                start=False,
                                stop=(si == len(TAP_SINGLES) - 1))
                    if b == 1:
                        nc.scalar.copy(stk[b][:], tp[:])
                    else:
                        nc.vector.tensor_copy(stk[b][:], tp[:])
                for p in range(2):
                    cix = g * 4 + 2 * p
                    first = (s == 0 and cix == 0)
                    last = (s == NS - 1 and cix == nchunk - 2)
                    c0 = (2 * p) * 128
                    sp = [stk[b][:, c0:c0 + 256].rearrange(
                        "p (two c) -> p two c", two=2) for b in range(3)]
                    nc.tensor.matmul(gA_ps[:, 0:128], sp[0], sp[0],
                                     start=first, stop=last, perf_mode=DR)
                    nc.tensor.matmul(gA_ps[:, 128:256], sp[1], sp[1],
                                     start=first, stop=last, perf_mode=DR)
                    nc.tensor.matmul(gA_ps[:, 256:384], sp[2], sp[2],
                                     start=first, stop=last, perf_mode=DR)
                    for h in range(HEADS):
                        for (lt, llo, lhi, roff) in _qpieces(h):
                            for (rt, rlo, rhi, coff) in _kpieces(h):
                                nc.tensor.matmul(
                                    _hout(h, roff, lhi - llo, coff,
                                          rhi - rlo),
                                    sp[lt][:, :, llo:lhi],
                                    sp[rt][:, :, rlo:rhi],
                                    start=first, stop=last, perf_mode=DR)

        # ---- norms from self-gram diagonals; per-head blocks -> SBUF
        g0m = small.tile([128, 128], F32, tag="g0m")
        nc.vector.tensor_tensor(g0m[:], gA_ps[:, 0:128], eye_t[:], ALU.mult)
        g1m = small.tile([128, 128], F32, tag="g1m")
        nc.vector.tensor_tensor(g1m[:], gA_ps[:, 128:256], eye_t[:],
                                ALU.mult)
        g2m = small.tile([128, 128], F32, tag="g2m")
        nc.vector.tensor_tensor(g2m[:], gA_ps[:, 256:384], eye_t[:],
                                ALU.mult)
        hAB_sb = small.tile([24, 216], F32, tag="hab")
        nc.scalar.copy(hAB_sb[:], hAB_ps[:])

    # ================= PASS 2 =========================================
    with tc.tile_pool(name="p2ps", bufs=1, space="PSUM") as pps:
        vb_tiles = {}

        def p2_vbdw(s):
            # v[120:192] depthwise on DVE (channel-major), PE does attn only
            tv_b = tv_tiles[s][1]
            tshb = tbuf.tile([72, LT], BF16, tag="tshb", name=f"tshb_{s}")
            nc.vector.tensor_copy(tshb[:, 0:LT - 2], tv_b[:, 1:LT - 1])
            vb = dwo.tile([72, SPXO], BF16, tag=f"vbdw{s % 2}",
                          name=f"vbdw_{s}", bufs=1)
            vb_tiles[s] = vb
            prodb = dwo.tile([72, SPXO], BF16, tag="prodb", name=f"prodb_{s}")
            vbv = vb[:].rearrange("p (r w) -> p r w", w=W)
            prodbv = prodb[:].rearrange("p (r w) -> p r w", w=W)
            for ti, (dy, dx) in enumerate(TAPS):
                sc = dvw2_t[:, ti:ti + 1]
                if dx == 1:
                    s3 = tv_b[:].rearrange("p (r w) -> p r w", w=WS)
                    view = s3[:, dy:dy + RS, 2:130]
                else:
                    s3 = tshb[:].rearrange("p (r w) -> p r w", w=WS)
                    view = s3[:, dy:dy + RS, dx:dx + 128]
                dstv = vbv if ti == 0 else prodbv
                nc.vector.tensor_scalar(dstv, view, sc, None, ALU.mult)
                if ti > 0:
                    nc.vector.tensor_tensor(vb[:], vb[:], prodb[:], ALU.add)

        def p2_attn(s):
            o0 = s * SPXO
            tv_a, tv_b = tv_tiles.pop(s)
            tva3 = tv_a[:].rearrange("p (r w) -> p r w", w=WS)
            vb = vb_tiles.pop(s)
            oa = outsb.tile([128, SPXO], F32, tag="oa", name=f"oa_{s}",
                            bufs=2)
            ob = outsb.tile([64, SPXO], F32, tag="ob", name=f"ob_{s}",
                            bufs=2)
            for n0, n in _chunks(SPXO, MMCH):
                r0, nr = n0 // W, n // W
                ops1 = pps.tile([120, MMCH], F32, tag="ops1", bufs=2)
                ops2 = pps.tile([72, MMCH], F32, tag="ops2", bufs=2)
                for ti, (dy, dx) in enumerate(TAPS):
                    nc.tensor.matmul(
                        ops1[:, 0:n], bd1t[:, ti * 120:(ti + 1) * 120],
                        tva3[0:120, r0 + dy:r0 + dy + nr, 1 + dx:129 + dx],
                        start=(ti == 0), stop=(ti == 8))
                nc.tensor.matmul(ops2[:, 0:n], bd2[:], vb[:, n0:n0 + n],
                                 start=True, stop=True)
                ao1 = dwo.tile([120, MMCH], BF16, tag="ao1", bufs=2)
                ao2 = dwo.tile([72, MMCH], BF16, tag="ao2", bufs=2)
                nc.scalar.copy(ao1[:, 0:n], ops1[:, 0:n])
                nc.scalar.copy(ao2[:, 0:n], ops2[:, 0:n])
                ppa = pps.tile([128, MMCH], F32, tag="ppa", bufs=2)
                ppb = pps.tile([64, MMCH], F32, tag="ppb", bufs=2)
                nc.tensor.matmul(ppa[:, 0:n], wp1_t[:, 0:128], ao1[:, 0:n],
                                 start=True, stop=False)
                nc.tensor.matmul(ppa[:, 0:n], wp2_t[:, 0:128], ao2[:, 0:n],
                                 start=False, stop=True)
                nc.tensor.matmul(ppb[:, 0:n], wp1_t[:, 128:192], ao1[:, 0:n],
                                 start=True, stop=False)
                nc.tensor.matmul(ppb[:, 0:n], wp2_t[:, 128:192], ao2[:, 0:n],
                                 start=False, stop=True)
                nc.scalar.copy(oa[:, n0:n0 + n], ppa[:, 0:n])
                nc.scalar.copy(ob[:, n0:n0 + n], ppb[:, 0:n])
                eng = nc.sync if (s == 3 and n0 + n == SPXO) else nc.gpsimd
                eng.dma_start(
                    io["outp"][0:128, o0 + n0:o0 + n0 + n], oa[:, n0:n0 + n])
                eng.dma_start(
                    io["outp"][128:192, o0 + n0:o0 + n0 + n],
                    ob[:, n0:n0 + n])

        # deferred conv-v for stripes 2,3: fills PE during the collective
        def p2_conv(s):
            tv_a, tv_b = tv_tiles[s]
            xa16, xb16 = x_tiles.pop(s)
            for n0, n in _chunks(SPXI, MMCH):
                r0, nr = n0 // W, n // W
                cv0 = pps.tile([120, MMCH], F32, tag="ops1", bufs=2,
                               name=f"dcv0_{s}_{n0}")
                cv1 = pps.tile([72, MMCH], F32, tag="ops2", bufs=2,
                               name=f"dcv1_{s}_{n0}")
                nc.tensor.matmul(cv0[:, 0:n], wv_a[:, 0:120],
                                 xa16[:, n0:n0 + n], start=True, stop=False)
                nc.tensor.matmul(cv0[:, 0:n], wv_b[:, 0:120],
                                 xb16[:, n0:n0 + n], start=False, stop=True)
                nc.tensor.matmul(cv1[:, 0:n], wv_a[:, 120:192],
                                 xa16[:, n0:n0 + n], start=True, stop=False)
                nc.tensor.matmul(cv1[:, 0:n], wv_b[:, 120:192],
                                 xb16[:, n0:n0 + n], start=False, stop=True)
                tvad = tv_a[:].rearrange("p (r w) -> p r w", w=WS)
                tvbd = tv_b[:].rearrange("p (r w) -> p r w", w=WS)
                nc.scalar.copy(
                    tvad[0:120, r0:r0 + nr, 2:130],
                    cv0[:, 0:n].rearrange("p (r w) -> p r w", w=W))
                nc.vector.tensor_copy(
                    tvbd[0:72, r0:r0 + nr, 2:130],
                    cv1[:, 0:n].rearrange("p (r w) -> p r w", w=W))

        qn_red = small.tile([128, 3], F32, tag="qnr")
        nc.vector.tensor_reduce(qn_red[:, 0:1], g0m[:], AX.X, ALU.add)
        nc.vector.tensor_reduce(qn_red[:, 1:2], g1m[:], AX.X, ALU.add)
        nc.vector.tensor_reduce(qn_red[:, 2:3], g2m[:], AX.X, ALU.add)
        # norm staging: na = [qn(q0:128) | kn(k0:128)], nb = rows 128:192
        na = small.tile([128, 2], F32, tag="na")
        nb = small.tile([64, 2], F32, tag="nb")
        nc.vector.tensor_copy(na[:, 0:1], qn_red[:, 0:1])
        nc.scalar.copy(na[0:64, 1:2], qn_red[64:128, 1:2])
        nc.scalar.copy(na[64:128, 1:2], qn_red[0:64, 2:3])
        nc.vector.tensor_copy(nb[:, 0:1], qn_red[0:64, 1:2])
        nc.scalar.copy(nb[:, 1:2], qn_red[64:128, 2:3])

        bounce_in = drm.tile([C, 26], F32)
        bounce_out = drm.tile([2 * C, 26], F32)
        nc.sync.dma_start(
            bounce_in[0:120, 0:CH].rearrange("(h c) k -> c h k", c=CH),
            hAB_sb[:, 0:120].rearrange("c (h k) -> c h k", h=5))
        nc.sync.dma_start(bounce_in[120:128, 0:CH], hAB_sb[0:8, 120:144])
        nc.sync.dma_start(bounce_in[128:144, 0:CH], hAB_sb[0:16, 192:216])
        nc.sync.dma_start(
            bounce_in[144:192, 0:CH].rearrange("(h c) k -> c h k", c=CH),
            hAB_sb[:, 144:192].rearrange("c (h k) -> c h k", h=2))
        nc.sync.dma_start(bounce_in[0:128, 24:26], na[:])
        nc.sync.dma_start(bounce_in[128:192, 24:26], nb[:])

        nc.gpsimd.collective_compute(
            "AllGather", ALU.bypass,
            replica_groups=[[0, 1], [2, 3], [4, 5], [6, 7]],
            ins=[bounce_in[:].opt()], outs=[bounce_out[:].opt()])

        with tc.tile_wait_until(0.090):
            p2_conv(2)
            p2_conv(3)
        p2_vbdw(0)
        p2_vbdw(1)
        p2_vbdw(2)
        p2_vbdw(3)

        # one DMA pulls all 384 gathered rows as [128, 3, 26]; the local
        # add then combines row j*128+p blocks (64-aligned cross-base)
        cmp3 = small.tile([128, 3 * 26], F32, tag="cmp3")
        nc.sync.dma_start(
            cmp3[:].rearrange("p (j k) -> p j k", j=3),
            bounce_out[:].rearrange("(j p) k -> p j k", j=3))
        c3v = cmp3[:].rearrange("p (j k) -> p j k", j=3)
        cmp_a = small.tile([128, 26], F32, tag="cmpa")
        cmp_b = small.tile([64, 26], F32, tag="cmpb")
        tmp_ab = small.tile([128, 26], F32, tag="cmptmp")
        nc.vector.tensor_copy(tmp_ab[0:64, :], c3v[64:128, 1, :])
        nc.vector.tensor_copy(tmp_ab[64:128, :], c3v[0:64, 2, :])
        nc.vector.tensor_tensor(cmp_a[:], c3v[:, 0, :], tmp_ab[:], ALU.add)
        tmp_b = small.tile([64, 26], F32, tag="cmptmpb")
        nc.scalar.copy(tmp_b[:], c3v[64:128, 2, :])
        nc.vector.tensor_tensor(cmp_b[:], c3v[0:64, 1, :], tmp_b[:],
                                ALU.add)

        kn8 = small.tile([HEADS, CH], F32, tag="kn8")
        kn8b = small.tile([HEADS, CH], F32, tag="kn8x")
        nc.gpsimd.dma_start(
            kn8[:],
            bounce_out[0:C, :].rearrange("(h c) k -> h c k", c=CH)[:, :, 25])
        nc.gpsimd.dma_start(
            kn8b[:],
            bounce_out[C:2 * C, :].rearrange("(h c) k -> h c k",
                                             c=CH)[:, :, 25])
        nc.vector.tensor_tensor(kn8[:], kn8[:], kn8b[:], ALU.add)

        # rq = temp/sqrt(qn); rk = 1/sqrt(kn) as [8,24]
        rq_a = small.tile([128, 3], F32, tag="rqa")
        rq_b = small.tile([64, 3], F32, tag="rqb")
        for ti, (cmp, rq, nrow) in enumerate(((cmp_a, rq_a, 128),
                                              (cmp_b, rq_b, 64))):
            nc.scalar.activation(rq[:, 0:1], cmp[:, 24:25], ACTF.Sqrt)
            nc.vector.reciprocal(rq[:, 1:2], rq[:, 0:1])
            nc.vector.tensor_scalar(rq[:, 2:3], rq[:, 1:2],
                                    tmpq_t[0:nrow, ti:ti + 1], None, ALU.mult)
        rk8 = small.tile([HEADS, 2 * CH], F32, tag="rk8")
        nc.scalar.activation(rk8[:, 0:CH], kn8[:], ACTF.Sqrt)
        nc.vector.reciprocal(rk8[:, CH:2 * CH], rk8[:, 0:CH])

        knb_a = pps.tile([128, CH], F32, tag="ppa", name="knb_ps", bufs=2)
        nc.tensor.matmul(knb_a[:], em_t[:, 0:128], rk8[:, CH:2 * CH],
                         start=True, stop=True)
        knb_b = pps.tile([128, CH], F32, tag="ppa", name="knb_ps2", bufs=2)
        nc.tensor.matmul(knb_b[0:64, :], em_t[:, 128:192],
                         rk8[:, CH:2 * CH], start=True, stop=True)

        # s = A*rq*knb ; softmax over d (free dim)
        attn16 = small.tile([128, CH], BF16, tag="att16a")
        attn16b = small.tile([64, CH], BF16, tag="att16b")
        for cmp, rq, knb, a16, nrow in (
                (cmp_a, rq_a, knb_a[:], attn16, 128),
                (cmp_b, rq_b, knb_b[0:64, :], attn16b, 64)):
            at = small.tile([128, CH], F32, tag="atf")
            sm = small.tile([128, 4], F32, tag="sm")
            nc.vector.tensor_scalar(at[0:nrow, :], cmp[0:nrow, 0:CH], rq[:, 2:3],
                                    None, ALU.mult)
            nc.vector.tensor_tensor(at[0:nrow, :], at[0:nrow, :], knb,
                                    ALU.mult)
            nc.scalar.activation(at[0:nrow, :], at[0:nrow, :], ACTF.Exp)
            nc.vector.tensor_reduce(sm[0:nrow, 1:2], at[0:nrow, :], AX.X, ALU.add)
            nc.vector.reciprocal(sm[0:nrow, 2:3], sm[0:nrow, 1:2])
            nc.vector.tensor_scalar(a16[0:nrow, :], at[0:nrow, :],
                                    sm[0:nrow, 2:3], None, ALU.mult)

        # block-diag attn^T via DRAM round-trip (transposing strided DMAs),
        # split across the HWDGE (sync) and SWDGE (gpsimd) queues
        attn_d = drm.tile([C, CH], BF16)
        nc.sync.dma_start(attn_d[0:128, :], attn16[:])
        nc.gpsimd.dma_start(attn_d[128:192, :], attn16b[:])
        bd1 = small.tile([120, 120], BF16, tag="bd1")      # heads 0-4 attn^T
        bd2 = small.tile([72, 72], BF16, tag="bd2")        # heads 5-7 attn^T
        nc.vector.memset(bd1[:], 0.0)
        nc.vector.memset(bd2[:], 0.0)
        for h in range(5):
            r0 = h * CH
            nc.sync.dma_start(
                bd1[r0:r0 + CH, r0:r0 + CH],
                attn_d[r0:r0 + CH, :].rearrange("c d -> d c"))
        for h in range(5, 8):
            r0 = (h - 5) * CH
            nc.gpsimd.dma_start(
                bd2[r0:r0 + CH, r0:r0 + CH],
                attn_d[h * CH:(h + 1) * CH, :].rearrange("c d -> d c"))
        # fold depthwise-v tap weights into attn^T: bd1_tap = bd1 * w_v[d,t]
        bd1t = small.tile([120, 9 * 120], BF16, tag="bd1t")
        for ti in range(9):
            nc.vector.tensor_scalar(bd1t[:, ti * 120:(ti + 1) * 120], bd1[:],
                                    dvw1_t[:, ti:ti + 1], None, ALU.mult)

        p2_attn(0)
        p2_attn(1)
        p2_attn(2)
        p2_attn(3)


# ======================================================================
def _prep_inputs(x, y, qkv_w, dw_w, proj_w, temperature):
    wq_t = np.ascontiguousarray(qkv_w[0:C].T)          # [in, out]
    wk_t = np.ascontiguousarray(qkv_w[C:2 * C].T)
    wv_t = np.ascontiguousarray(qkv_w[2 * C:3 * C].T)
    wp_t = np.ascontiguousarray(proj_w.T)

    def planes2(w):
        out = np.zeros((128, 2, C), np.float32)
        out[:, 0, :] = w[0:128]
        out[0:64, 1, :] = w[128:192]
        return out.reshape(128, 2 * C).astype(F8NP)

    wq8, wk8 = planes2(wq_t), planes2(wk_t)
    wv16 = wv_t.astype(BF16NP)
    wp1 = wp_t[0:120].astype(BF16NP)
    wp2 = wp_t[120:192].astype(BF16NP)

    dw = dw_w.reshape(3 * C, 9).astype(np.float32)
    dw_q, dw_k, dw_v = dw[0:C], dw[C:2 * C], dw[2 * C:3 * C]
    dqk = np.concatenate([dw_q[0:128], dw_q[128:192], dw_k[0:64],
                          dw_k[64:192]], axis=0)
    dqkd = np.zeros((3 * 128, 9 * 128), np.float32)
    for i in range(3):
        for t in range(9):
            blk = dqk[i * 128:(i + 1) * 128, t]
            np.fill_diagonal(
                dqkd[i * 128:(i + 1) * 128, t * 128:(t + 1) * 128], blk)
    tmpq = np.repeat(np.asarray(temperature, np.float32).reshape(HEADS),
                     CH).reshape(C, 1)
    em = np.zeros((HEADS, C), np.float32)
    for hh in range(HEADS):
        em[hh, hh * CH:(hh + 1) * CH] = 1.0

    in_maps = []
    for core in range(8):
        bi, half = core // 2, core % 2
        r0 = half * HOUT - 1
        xsl = np.zeros((C, HIN, W), np.float32)
        ysl = np.zeros((C, HIN, W), np.float32)
        lo, hi = max(r0, 0), min(r0 + HIN, 128)
        xsl[:, lo - r0:hi - r0] = x[bi, :, lo:hi]
        ysl[:, lo - r0:hi - r0] = y[bi, :, lo:hi]
        xsl = xsl.reshape(C, PXIN)
        ysl = ysl.reshape(C, PXIN)

        def planes_px(t):
            out = np.zeros((128, 2, PXIN), np.float32)
            out[:, 0, :] = t[0:128]
            out[0:64, 1, :] = t[128:192]
            return out.reshape(128, 2 * PXIN).astype(F8NP)

        in_maps.append({
            "xs": xsl.astype(BF16NP),
            "xs8": planes_px(xsl), "ys8": planes_px(ysl),
            "wq8": wq8, "wk8": wk8, "wv": wv16,
            "wp1": wp1, "wp2": wp2,
            "dqkd": dqkd.astype(F8NP),
            "dvw": dw_v.astype(np.float32),
            "tmpq": tmpq, "em": em,
            "eye": np.eye(128, dtype=np.float32),
        })
    return in_maps


def kernel(x, y, qkv_w, dw_w, proj_w, temperature, _trace=False):
    x = np.asarray(x, np.float32)
    y = np.asarray(y, np.float32)
    if "nc" not in _CACHE:
        _CACHE["nc"] = build_program()
    nc = _CACHE["nc"]
    in_maps = _prep_inputs(x, y, np.asarray(qkv_w, np.float32),
                           np.asarray(dw_w, np.float32),
                           np.asarray(proj_w, np.float32),
                           np.asarray(temperature, np.float32))
    res = bass_utils.run_bass_kernel_spmd(nc, in_maps,
                                          core_ids=list(range(8)),
                                          trace=_trace)
    _CACHE["last_result"] = res
    out = np.empty((4, C, 128, W), np.float32)
    for core in range(8):
        bi, half = core // 2, core % 2
        out[bi, :, half * HOUT:(half + 1) * HOUT] = \
            res.results[core]["outp"].reshape(C, HOUT, W)
    return out
